# revision 15
# baseline (speedup 1.0000x reference)
"""DualDeltaBlock Trainium2 kernel: 8-core SPMD, head-parallel sharding.

Core c handles head h=c for both batch rows (2 (b,h) pairs per core).
Algorithm (validated in numpy golden model, max scaled err 8.9e-7):
  - fp32r projections x@W (TF32-like, 1cyc/row @ N>=256)
  - C=64 chunked dual-delta scan, WY inverse via 3-factor product form
    A = (I+M)(I+M^2)(I+M^4), two chunks batched block-diagonally per 128-wide
    matmul group
  - decay factors folded into row/column scalings (chunk-end referenced)
  - per-head partial out-projection; host sums partials + residual
"""
import numpy as np

B, T, D, H = 2, 2048, 1024, 8
HD = 128
C = 64
N = T // C          # 32 chunks / pair
NT = T // 128       # 16 token tiles (= WY groups)
ND = D // 128       # 8 feature blocks
NSLOT = 16

_PROG = None


def _build_program():
    import concourse.bass as bass
    import concourse.mybir as mybir
    import concourse.tile as tile
    from concourse import bacc

    F32 = mybir.dt.float32
    F32R = mybir.dt.float32r
    BF16 = mybir.dt.bfloat16
    AF = mybir.ActivationFunctionType
    OP = mybir.AluOpType

    nc = bacc.Bacc("TRN2", target_bir_lowering=False, debug=False)
    xT = nc.declare_dram_parameter("xT", [B, D, T], F32R, isOutput=False)
    Wv = nc.declare_dram_parameter("Wv", [D, 256], F32R, isOutput=False)
    Wsc = nc.declare_dram_parameter("Wsc", [D, 8], F32R, isOutput=False)
    WoT = nc.declare_dram_parameter("WoT", [HD, D], F32R, isOutput=False)
    consts = nc.declare_dram_parameter("consts", [1, 4], F32, isOutput=False)
    maskL = nc.declare_dram_parameter("maskL", [128, 128], F32, isOutput=False)
    maskU = nc.declare_dram_parameter("maskU", [128, 128], F32, isOutput=False)
    maskUi = nc.declare_dram_parameter("maskUi", [128, 128], F32, isOutput=False)
    ident = nc.declare_dram_parameter("ident", [128, 128], F32, isOutput=False)
    part = nc.declare_dram_parameter("part", [B, D, T], F32, isOutput=True)
    scr = nc.dram_tensor("scr", [B, NSLOT, T], F32)
    scrg = nc.dram_tensor("scrg", [B, 2, N], F32)

    with tile.TileContext(nc) as tc:
        with tc.tile_pool(name="const", bufs=1) as constp, \
             tc.tile_pool(name="big", bufs=1) as bigp, \
             tc.tile_pool(name="work", bufs=2) as workp, \
             tc.tile_pool(name="bc", bufs=2) as bcp, \
             tc.tile_pool(name="wyt", bufs=2) as wytp, \
             tc.tile_pool(name="wyo", bufs=3) as wyop, \
             tc.tile_pool(name="sc", bufs=1) as scp, \
             tc.tile_pool(name="ot", bufs=3) as otp, \
             tc.tile_pool(name="pp", bufs=2, space="PSUM") as ppp, \
             tc.tile_pool(name="pw", bufs=2, space="PSUM") as pwp, \
             tc.tile_pool(name="ps", bufs=1, space="PSUM") as psp:

            mL = constp.tile([128, 128], F32, tag="mL")
            mU = constp.tile([128, 128], F32, tag="mU")
            mUi = constp.tile([128, 128], F32, tag="mUi")
            idn = constp.tile([128, 128], F32, tag="idn")
            idnR = constp.tile([128, 128], F32R, tag="idnR")
            cst = constp.tile([128, 4], F32, tag="cst")
            nc.gpsimd.dma_start(out=mL, in_=maskL.ap())
            nc.gpsimd.dma_start(out=mU, in_=maskU.ap())
            nc.gpsimd.dma_start(out=mUi, in_=maskUi.ap())
            nc.gpsimd.dma_start(out=idn, in_=ident.ap())
            nc.vector.tensor_copy(out=idnR, in_=idn)
            nc.gpsimd.dma_start(out=cst, in_=consts.ap().to_broadcast([128, 4]))
            Wvt = constp.tile([128, ND, 256], F32R, tag="Wvt")
            nc.gpsimd.dma_start(out=Wvt,
                                in_=Wv.ap().rearrange("(k p) n -> p k n", p=128))
            Wst = constp.tile([128, ND, 8], F32R, tag="Wst")
            nc.gpsimd.dma_start(out=Wst,
                                in_=Wsc.ap().rearrange("(k p) n -> p k n", p=128))
            Wot = constp.tile([HD, D], F32R, tag="Wot")
            nc.gpsimd.dma_start(out=Wot, in_=WoT.ap())
            one32 = constp.tile([128, 1], F32, tag="one32")
            nc.vector.memset(one32, 1.0)
            one_lhs = constp.tile([128, 1], F32R, tag="onec")
            nc.vector.tensor_copy(out=one_lhs, in_=one32)

            for b in range(B):
                # ---------- load xT ----------
                xtiles = []
                for k in range(ND):
                    xt = bigp.tile([128, T], F32R, tag=f"x{k}")
                    nc.gpsimd.dma_start(out=xt, in_=xT.ap()[b, k*128:(k+1)*128, :])
                    xtiles.append(xt)
                xh = xtiles[0]  # host puts this head's own feature block first

                # ---------- projections ----------
                b1t = scp.tile([128, NT], F32, tag="b1t")
                b2t = scp.tile([128, NT], F32, tag="b2t")
                gt = scp.tile([128, NT], F32, tag="gt")
                d1t = scp.tile([128, NT], F32, tag="d1t")
                d2t = scp.tile([128, NT], F32, tag="d2t")
                vT = []
                for tt in range(NT):
                    pv = ppp.tile([128, 256], F32, tag="pp")
                    psc = psp.tile([128, 8], F32, tag="pso")
                    for k in range(ND):
                        st, sp_ = (k == 0), (k == ND - 1)
                        nc.tensor.matmul(pv,
                                         lhsT=xtiles[k][:, tt*128:(tt+1)*128],
                                         rhs=Wvt[:, k, :], start=st, stop=sp_)
                        nc.tensor.matmul(psc,
                                         lhsT=xtiles[k][:, tt*128:(tt+1)*128],
                                         rhs=Wst[:, k, :], start=st, stop=sp_)
                    nc.scalar.activation(out=gt[:, tt:tt+1], in_=psc[:, 0:1],
                                         func=AF.Sigmoid)
                    nc.scalar.activation(out=b1t[:, tt:tt+1], in_=psc[:, 1:2],
                                         func=AF.Sigmoid)
                    nc.scalar.activation(out=b2t[:, tt:tt+1], in_=psc[:, 2:3],
                                         func=AF.Sigmoid)
                    sp1 = workp.tile([128, 2], F32, tag="sp1")
                    # softplus(x+db) = -ln(sigmoid(-x-db)); dec=-a*softplus
                    # => dec = a*ln(sigmoid(-x-db)); cst holds [-db1,-db2,a1,a2]
                    nc.scalar.activation(out=sp1[:, 0:1], in_=psc[:, 3:4],
                                         func=AF.Sigmoid, scale=-1.0,
                                         bias=cst[:, 0:1])
                    nc.scalar.activation(out=sp1[:, 1:2], in_=psc[:, 4:5],
                                         func=AF.Sigmoid, scale=-1.0,
                                         bias=cst[:, 1:2])
                    nc.scalar.activation(out=sp1[:, 0:1], in_=sp1[:, 0:1],
                                         func=AF.Ln)
                    nc.scalar.activation(out=sp1[:, 1:2], in_=sp1[:, 1:2],
                                         func=AF.Ln)
                    nc.vector.tensor_scalar_mul(out=d1t[:, tt:tt+1],
                                                in0=sp1[:, 0:1],
                                                scalar1=cst[:, 2:3])
                    nc.vector.tensor_scalar_mul(out=d2t[:, tt:tt+1],
                                                in0=sp1[:, 1:2],
                                                scalar1=cst[:, 3:4])
                    vt = scp.tile([128, 256], F32, tag=f"vT{tt}")
                    nc.vector.tensor_scalar_mul(out=vt[:, 0:128],
                                                in0=pv[:, 0:128],
                                                scalar1=b1t[:, tt:tt+1])
                    nc.vector.tensor_scalar_mul(out=vt[:, 128:256],
                                                in0=pv[:, 128:256],
                                                scalar1=b2t[:, tt:tt+1])
                    vT.append(vt)

                # ---------- rnorm sumsq ----------
                xsq = bcp.tile([128, T], F32R, tag="bct")
                nc.vector.tensor_mul(out=xsq, in0=xh, in1=xh)
                sq_s = bcp.tile([1, T], F32, tag="bct")
                for q in range(4):
                    pq = pwp.tile([1, 512], F32, tag="pw")
                    nc.tensor.matmul(pq, lhsT=one_lhs,
                                     rhs=xsq[:, q*512:(q+1)*512],
                                     start=True, stop=True)
                    nc.vector.tensor_copy(out=sq_s[:, q*512:(q+1)*512], in_=pq)
                nc.gpsimd.dma_start(out=scr.ap()[b, 4, :], in_=sq_s[0:1, :])

                # ---------- bounce tables, chunk-row math ----------
                def tab_to_dram(tab, slot):
                    dst = scr.ap()[b, slot, :].rearrange("(k p) -> p k", p=128)
                    nc.gpsimd.dma_start(out=dst, in_=tab)
                tab_to_dram(d1t, 0)
                tab_to_dram(d2t, 1)
                tab_to_dram(b1t, 2)
                tab_to_dram(b2t, 3)

                def cr_load(slot, tag):
                    t = workp.tile([N, C], F32, tag=tag)
                    nc.gpsimd.dma_start(
                        out=t,
                        in_=scr.ap()[b, slot, :].rearrange("(n c) -> n c", c=C))
                    return t
                d1r = cr_load(0, "d1r")
                d2r = cr_load(1, "d2r")
                b1r = cr_load(2, "b1r")
                b2r = cr_load(3, "b2r")
                rnr = cr_load(4, "rnr")
                nc.scalar.activation(out=rnr, in_=rnr, func=AF.Sqrt)
                nc.vector.reciprocal(out=rnr, in_=rnr)

                c1r = workp.tile([N, C], F32, tag="c1r")
                c2r = workp.tile([N, C], F32, tag="c2r")
                cvr = workp.tile([N, C], F32, tag="cvr")
                nc.vector.tensor_tensor_scan(out=c1r, data0=d1r, data1=d1r,
                                             initial=0.0, op0=OP.add,
                                             op1=OP.bypass)
                nc.vector.tensor_tensor_scan(out=c2r, data0=d2r, data1=d2r,
                                             initial=0.0, op0=OP.add,
                                             op1=OP.bypass)
                nc.vector.tensor_add(out=cvr, in0=c1r, in1=c2r)
                nc.scalar.mul(out=cvr, in_=cvr, mul=0.5)
                nlast = workp.tile([N, 3], F32, tag="nlast")
                nc.scalar.mul(out=nlast[:, 0:1], in_=c1r[:, C-1:C], mul=-1.0)
                nc.scalar.mul(out=nlast[:, 1:2], in_=c2r[:, C-1:C], mul=-1.0)
                nc.scalar.mul(out=nlast[:, 2:3], in_=cvr[:, C-1:C], mul=-1.0)

                rowt = {}
                def row(tag):
                    t = workp.tile([N, C], F32, tag="row_" + tag)
                    rowt[tag] = t
                    return t
                nc.scalar.activation(out=row("e1p"), in_=c1r, func=AF.Exp,
                                     bias=nlast[:, 0:1])
                nc.scalar.activation(out=row("e2p"), in_=c2r, func=AF.Exp,
                                     bias=nlast[:, 1:2])
                nc.scalar.activation(out=row("dw1"), in_=c1r, func=AF.Exp,
                                     scale=-1.0, bias=c1r[:, C-1:C])
                nc.scalar.activation(out=row("dw2"), in_=c2r, func=AF.Exp,
                                     scale=-1.0, bias=c2r[:, C-1:C])
                nc.scalar.activation(out=row("de1"), in_=c1r, func=AF.Exp)
                nc.scalar.activation(out=row("de2"), in_=c2r, func=AF.Exp)
                nc.scalar.activation(out=row("eavm"), in_=cvr, func=AF.Exp,
                                     scale=-1.0, bias=cvr[:, C-1:C])
                bavp = row("bavp")
                nc.scalar.activation(out=bavp, in_=cvr, func=AF.Exp,
                                     bias=nlast[:, 2:3])
                tmpb = workp.tile([N, C], F32, tag="tmpb")
                nc.vector.tensor_add(out=tmpb, in0=b1r, in1=b2r)
                nc.vector.tensor_mul(out=bavp, in0=bavp, in1=tmpb)
                nc.scalar.mul(out=bavp, in_=bavp, mul=-0.5)
                nc.vector.tensor_mul(out=row("r_e1p"), in0=rowt["e1p"], in1=rnr)
                nc.vector.tensor_mul(out=row("r_e2p"), in0=rowt["e2p"], in1=rnr)

                gcol = workp.tile([N, 2], F32, tag="gcol")
                nc.scalar.activation(out=gcol[:, 0:1], in_=c1r[:, C-1:C],
                                     func=AF.Exp)
                nc.scalar.activation(out=gcol[:, 1:2], in_=c2r[:, C-1:C],
                                     func=AF.Exp)
                nc.gpsimd.dma_start(out=scrg.ap()[b, 0, :], in_=gcol[:, 0:1])
                nc.gpsimd.dma_start(out=scrg.ap()[b, 1, :], in_=gcol[:, 1:2])
                gtab = scp.tile([128, 2 * N], F32, tag="gtab")
                nc.gpsimd.dma_start(
                    out=gtab[:, 0:N],
                    in_=scrg.ap()[b, 0:1, :].to_broadcast([128, N]))
                nc.gpsimd.dma_start(
                    out=gtab[:, N:2*N],
                    in_=scrg.ap()[b, 1:2, :].to_broadcast([128, N]))

                slot_of = {"r_e1p": 5, "r_e2p": 6, "eavm": 7, "bavp": 8,
                           "de1": 9, "de2": 10, "dw1": 11, "dw2": 12}
                for tag, slot in slot_of.items():
                    nc.gpsimd.dma_start(
                        out=scr.ap()[b, slot, :].rearrange("(n c) -> n c", c=C),
                        in_=rowt[tag])
                nc.gpsimd.dma_start(
                    out=scr.ap()[b, 13, :].rearrange("(n c) -> n c", c=C),
                    in_=rnr)

                def tmaj(slot, tag):
                    t = scp.tile([128, NT], F32, tag=tag)
                    nc.gpsimd.dma_start(
                        out=t, in_=scr.ap()[b, slot, :].rearrange(
                            "(k p) -> p k", p=128))
                    return t
                de1t = tmaj(9, "de1t")
                de2t = tmaj(10, "de2t")
                dw1t = tmaj(11, "dw1t")
                dw2t = tmaj(12, "dw2t")
                de1g = scp.tile([128, NT], F32, tag="de1g")
                de2g = scp.tile([128, NT], F32, tag="de2g")
                nc.vector.tensor_mul(out=de1g, in0=de1t, in1=gt)
                nc.vector.tensor_mul(out=de2g, in0=de2t, in1=gt)
                bde1 = scp.tile([128, NT], F32, tag="bde1")
                bde2 = scp.tile([128, NT], F32, tag="bde2")
                nc.vector.tensor_mul(out=bde1, in0=b1t, in1=de1t)
                nc.scalar.mul(out=bde1, in_=bde1, mul=-1.0)
                nc.vector.tensor_mul(out=bde2, in0=b2t, in1=de2t)
                nc.scalar.mul(out=bde2, in_=bde2, mul=-1.0)

                # ---------- F-major builds ----------
                def bcast(slot):
                    t = bcp.tile([128, T], F32, tag="bct")
                    nc.gpsimd.dma_start(
                        out=t,
                        in_=scr.ap()[b, slot:slot+1, :].to_broadcast([128, T]))
                    return t
                xh32 = xh.bitcast(F32)
                rkF = bigp.tile([128, T + 4], F32, tag="rkF")
                nc.vector.memset(rkF[:, 0:1], 0.0)
                bcr = bcast(13)
                nc.vector.tensor_mul(out=rkF[:, 1:T+1], in0=xh32, in1=bcr)
                rk1 = bigp.tile([128, T], F32, tag="rk1")
                bc1 = bcast(5)
                nc.vector.tensor_mul(out=rk1, in0=xh32, in1=bc1)
                rk2 = bigp.tile([128, T], F32, tag="rk2")
                bc2 = bcast(6)
                nc.vector.tensor_mul(out=rk2, in0=xh32, in1=bc2)
                wkM = bigp.tile([128, T], F32, tag="wkM")
                bc3 = bcast(7)
                nc.vector.tensor_mul(out=wkM, in0=rkF[:, 0:T], in1=bc3)
                wkP = bigp.tile([128, T], F32, tag="wkP")
                bc4 = bcast(8)
                nc.vector.tensor_mul(out=wkP, in0=rkF[:, 0:T], in1=bc4)

                # ---------- wk_T transposes ----------
                wkT = []
                for tt in range(NT):
                    pt = pwp.tile([128, 128], F32, tag="pw")
                    nc.tensor.transpose(pt, in_=rkF[:, tt*128:(tt+1)*128],
                                        identity=idn)
                    wt = scp.tile([128, 128], F32, tag=f"wkT{tt}")
                    nc.vector.tensor_copy(out=wt, in_=pt)
                    wkT.append(wt)

                # ---------- WY groups ----------
                wc1s, wc2s, in1s, in2s, vcs = [], [], [], [], []
                for g in range(NT):
                    wsl = slice(g*128, (g+1)*128)
                    usl = slice(g*128, (g+1)*128)
                    pm = pwp.tile([128, 128], F32, tag="pw")
                    nc.tensor.matmul(pm, lhsT=wkP[:, usl], rhs=wkM[:, usl],
                                     start=True, stop=True)
                    m_ = wytp.tile([128, 128], F32, tag="m_")
                    nc.vector.tensor_mul(out=m_, in0=pm, in1=mL)
                    pmt = pwp.tile([128, 128], F32, tag="pw")
                    nc.tensor.matmul(pmt, lhsT=wkM[:, usl], rhs=wkP[:, usl],
                                     start=True, stop=True)
                    mt = wytp.tile([128, 128], F32, tag="mt")
                    nc.vector.tensor_mul(out=mt, in0=pmt, in1=mU)
                    mtI = wytp.tile([128, 128], F32, tag="mtI")
                    nc.vector.tensor_add(out=mtI, in0=mt, in1=idn)
                    p2 = pwp.tile([128, 128], F32, tag="pw")
                    nc.tensor.matmul(p2, lhsT=mt, rhs=m_, start=True, stop=True)
                    m2I = wytp.tile([128, 128], F32, tag="m2I")
                    nc.vector.tensor_add(out=m2I, in0=p2, in1=idn)
                    m2 = wytp.tile([128, 128], F32, tag="m2")
                    nc.vector.tensor_copy(out=m2, in_=p2)
                    pt2 = pwp.tile([128, 128], F32, tag="pw")
                    nc.tensor.matmul(pt2, lhsT=m_, rhs=mt, start=True, stop=True)
                    mt2 = wytp.tile([128, 128], F32, tag="mt2")
                    nc.vector.tensor_copy(out=mt2, in_=pt2)
                    p4 = pwp.tile([128, 128], F32, tag="pw")
                    nc.tensor.matmul(p4, lhsT=mt2, rhs=m2, start=True, stop=True)
                    m4I = wytp.tile([128, 128], F32, tag="m4I")
                    nc.vector.tensor_add(out=m4I, in0=p4, in1=idn)
                    pp1 = pwp.tile([128, 128], F32, tag="pw")
                    nc.tensor.matmul(pp1, lhsT=m2I, rhs=mtI, start=True,
                                     stop=True)
                    p1t = wytp.tile([128, 128], F32, tag="p1t")
                    nc.vector.tensor_copy(out=p1t, in_=pp1)
                    pat = pwp.tile([128, 128], F32, tag="pw")
                    nc.tensor.matmul(pat, lhsT=m4I, rhs=p1t, start=True,
                                     stop=True)
                    at = wytp.tile([128, 128], F32, tag="at")
                    nc.vector.tensor_copy(out=at, in_=pat)
                    pvc = pwp.tile([128, 256], F32, tag="pw")
                    nc.tensor.matmul(pvc, lhsT=at, rhs=vT[g], start=True,
                                     stop=True)
                    vc = wyop.tile([128, 256], F32, tag="vc")
                    nc.vector.tensor_scalar_mul(out=vc[:, 0:128],
                                                in0=pvc[:, 0:128],
                                                scalar1=dw1t[:, g:g+1])
                    nc.vector.tensor_scalar_mul(out=vc[:, 128:256],
                                                in0=pvc[:, 128:256],
                                                scalar1=dw2t[:, g:g+1])
                    vcs.append(vc)
                    for bt_, lst, tg in ((bde1, wc1s, "wc1"), (bde2, wc2s, "wc2")):
                        xw = wytp.tile([128, 128], F32, tag="xw" + tg)
                        nc.vector.tensor_scalar_mul(out=xw, in0=wkT[g],
                                                    scalar1=bt_[:, g:g+1])
                        pwc = pwp.tile([128, 128], F32, tag="pw")
                        nc.tensor.matmul(pwc, lhsT=xw, rhs=at, start=True,
                                         stop=True)
                        wc = wyop.tile([128, 128], F32, tag=tg)
                        nc.vector.tensor_copy(out=wc, in_=pwc)
                        lst.append(wc)
                    for rkv, lst, tg in ((rk1, in1s, "in1"), (rk2, in2s, "in2")):
                        pig = pwp.tile([128, 128], F32, tag="pw")
                        nc.tensor.matmul(pig, lhsT=rkF[:, wsl], rhs=rkv[:, usl],
                                         start=True, stop=True)
                        ig = wyop.tile([128, 128], F32, tag=tg)
                        nc.vector.tensor_mul(out=ig, in0=pig, in1=mUi)
                        lst.append(ig)

                # ---------- scan ----------
                S = bigp.tile([128, 256], F32, tag="S")
                nc.vector.memset(S, 0.0)
                oF = bigp.tile([128, T], F32R, tag="oF")
                otile = None
                for n in range(N):
                    tt, half = n // 2, n % 2
                    hs = half * 64
                    he = hs + 64
                    if half == 0:
                        otile = otp.tile([128, 128], F32R, tag="ot")
                    vns = workp.tile([128, 256], F32, tag="vns")
                    pvn = psp.tile([128, 128], F32, tag="pvn")
                    nc.tensor.matmul(pvn[hs:he, :], lhsT=wc1s[tt][:, hs:he],
                                     rhs=S[:, 0:128], start=True, stop=True)
                    nc.vector.scalar_tensor_tensor(
                        out=vns[hs:he, 0:128], in0=pvn[hs:he, :],
                        scalar=dw1t[hs:he, tt:tt+1],
                        in1=vcs[tt][hs:he, 0:128], op0=OP.mult, op1=OP.add)
                    pvn2 = psp.tile([128, 128], F32, tag="pvn")
                    nc.tensor.matmul(pvn2[hs:he, :], lhsT=wc2s[tt][:, hs:he],
                                     rhs=S[:, 128:256], start=True, stop=True)
                    nc.vector.scalar_tensor_tensor(
                        out=vns[hs:he, 128:256], in0=pvn2[hs:he, :],
                        scalar=dw2t[hs:he, tt:tt+1],
                        in1=vcs[tt][hs:he, 128:256], op0=OP.mult, op1=OP.add)
                    pso = psp.tile([128, 256], F32, tag="pso")
                    nc.tensor.matmul(pso[hs:he, :], lhsT=rkF[:, n*64+1:n*64+65],
                                     rhs=S, start=True, stop=True)
                    pin = psp.tile([128, 128], F32, tag="pvn")
                    nc.tensor.matmul(pin[hs:he, :], lhsT=in1s[tt][hs:he, hs:he],
                                     rhs=vns[hs:he, 0:128], start=True,
                                     stop=False, skip_group_check=True)
                    nc.tensor.matmul(pin[hs:he, :], lhsT=in2s[tt][hs:he, hs:he],
                                     rhs=vns[hs:he, 128:256], start=False,
                                     stop=True, skip_group_check=True)
                    t1 = workp.tile([128, 128], F32, tag="t1")
                    nc.vector.tensor_scalar_mul(out=t1[hs:he, :],
                                                in0=pso[hs:he, 0:128],
                                                scalar1=de1g[hs:he, tt:tt+1])
                    t2 = workp.tile([128, 128], F32, tag="t2")
                    nc.vector.scalar_tensor_tensor(
                        out=t2[hs:he, :], in0=pso[hs:he, 128:256],
                        scalar=de2g[hs:he, tt:tt+1], in1=t1[hs:he, :],
                        op0=OP.mult, op1=OP.add)
                    nc.vector.scalar_tensor_tensor(
                        out=otile[hs:he, :], in0=pin[hs:he, :],
                        scalar=gt[hs:he, tt:tt+1], in1=t2[hs:he, :],
                        op0=OP.mult, op1=OP.add)
                    pu = psp.tile([128, 256], F32, tag="pu")
                    nc.tensor.matmul(pu, lhsT=wkT[tt][hs:he, :],
                                     rhs=vns[hs:he, :],
                                     start=True, stop=True)
                    nc.vector.scalar_tensor_tensor(
                        out=S[:, 0:128], in0=S[:, 0:128],
                        scalar=gtab[:, n:n+1], in1=pu[:, 0:128],
                        op0=OP.mult, op1=OP.add)
                    nc.vector.scalar_tensor_tensor(
                        out=S[:, 128:256], in0=S[:, 128:256],
                        scalar=gtab[:, N+n:N+n+1], in1=pu[:, 128:256],
                        op0=OP.mult, op1=OP.add)
                    if half == 1:
                        pot = ppp.tile([128, 128], F32R, tag="pp")
                        nc.tensor.transpose(pot, in_=otile, identity=idnR)
                        nc.vector.tensor_copy(out=oF[:, tt*128:(tt+1)*128],
                                              in_=pot)

                # ---------- out-projection ----------
                for db in range(ND):
                    for tq in range(4):
                        po = ppp.tile([128, 512], F32, tag="pp")
                        nc.tensor.matmul(po, lhsT=Wot[:, db*128:(db+1)*128],
                                         rhs=oF[:, tq*512:(tq+1)*512],
                                         start=True, stop=True)
                        ob = bcp.tile([128, 512], F32, tag="bct")
                        nc.vector.tensor_copy(out=ob, in_=po)
                        nc.gpsimd.dma_start(
                            out=part.ap()[b, db*128:(db+1)*128,
                                          tq*512:(tq+1)*512],
                            in_=ob)
    nc.compile()
    return nc


def _host_inputs(out, W_write, W_gate, W_out, W_b1, W_b2, W_a1, W_a2,
                 dt_bias1, dt_bias2, A_log1, A_log2):
    x = np.ascontiguousarray(out.astype(np.float32))
    i = np.arange(128)
    blk = (i[:, None] // 64) == (i[None, :] // 64)
    maskL = ((i[:, None] > i[None, :]) & blk).astype(np.float32)
    maskU = ((i[:, None] < i[None, :]) & blk).astype(np.float32)
    maskUi = ((i[:, None] <= i[None, :]) & blk).astype(np.float32)
    ident = np.eye(128, dtype=np.float32)
    xb_base = np.ascontiguousarray(x.transpose(0, 2, 1))  # (B, D, T)

    in_maps = []
    for h in range(H):
        perm = [h] + [k for k in range(ND) if k != h]
        xb = xb_base.reshape(B, ND, 128, T)[:, perm].reshape(B, D, T)
        Wv1 = W_write[h*HD:(h+1)*HD]
        Wv2 = W_write[D + h*HD: D + (h+1)*HD]
        WvT = np.concatenate([Wv1.T, Wv2.T], axis=1)   # (D, 256)
        Wsc = np.zeros((D, 8), np.float32)
        Wsc[:, 0] = W_gate[h]
        Wsc[:, 1] = W_b1[h]
        Wsc[:, 2] = W_b2[h]
        Wsc[:, 3] = W_a1[h]
        Wsc[:, 4] = W_a2[h]
        WvT = WvT.reshape(ND, 128, 256)[perm].reshape(D, 256)
        Wsc = Wsc.reshape(ND, 128, 8)[perm].reshape(D, 8)
        WoT = np.ascontiguousarray(W_out[:, h*HD:(h+1)*HD].T)
        consts = np.array([[-dt_bias1[h], -dt_bias2[h],
                            np.exp(A_log1[h]), np.exp(A_log2[h])]],
                          np.float32)
        in_maps.append({
            "xT": np.ascontiguousarray(xb),
            "Wv": np.ascontiguousarray(WvT.astype(np.float32)),
            "Wsc": Wsc,
            "WoT": WoT.astype(np.float32),
            "consts": consts,
            "maskL": maskL, "maskU": maskU, "maskUi": maskUi, "ident": ident,
        })
    return in_maps


def kernel(out, W_write, W_gate, W_out, W_b1, W_b2, W_a1, W_a2,
           dt_bias1, dt_bias2, A_log1, A_log2):
    global _PROG
    from concourse import bass_utils
    args = [np.asarray(a) for a in (out, W_write, W_gate, W_out, W_b1, W_b2,
                                    W_a1, W_a2, dt_bias1, dt_bias2,
                                    A_log1, A_log2)]
    in_maps = _host_inputs(*args)
    if _PROG is None:
        _PROG = _build_program()
    res = bass_utils.run_bass_kernel_spmd(_PROG, in_maps,
                                          core_ids=list(range(H)))
    acc = np.zeros((B, D, T), np.float32)
    for h in range(H):
        acc += res.results[h]["part"]
    final = args[0].astype(np.float32) + acc.transpose(0, 2, 1)
    return final.astype(np.float32)
